# revision 13
# baseline (speedup 1.0000x reference)
"""GQA attention layer (dense_transformer) on 8 Trainium2 NeuronCores.

Sharding: data-parallel over batch (2) x tensor-parallel over head groups (4).
Core c handles batch c//4 and head-group c%4 (8 q heads, 2 kv heads).
Each core computes a partial output (its heads' contribution through its
Wo row-slice); the host sums the 4 partials per batch.

Per-core pipeline (all matmuls bf16, fp32 accumulation):
  P1a: q = hs @ Wq_shard  -> RMSNorm -> RoPE -> transpose -> qT [d, i]
  P1b: k,v = hs @ Wkv_shard -> (k: norm+rope+transpose -> kT), (v: v_aug
       token-major with a ones column appended for softmax denominators)
  P2:  per head: scoresT[j,i] = kT_tile.T @ qT (pre-scaled via tables),
       exp on ScalarE, causal mask on diagonal tiles, PV with v_aug gives
       attn_out[i,d] AND row sums in column 128; normalize by reciprocal
       of the sums during eviction; transpose -> aoT [d, i]
  P3:  out_partial = aoT.T @ Wo_shard
"""
import math
import os
import sys
from contextlib import ExitStack

import numpy as np

_REPO = "/opt/trn_rl_repo"
_PKGS = "/opt/pypackages"
for _p in (_REPO, _PKGS):
    if _p not in sys.path:
        sys.path.append(_p)

import ml_dtypes

BF16 = ml_dtypes.bfloat16

B, S, HIDDEN = 2, 2048, 4096
NUM_HEADS, NUM_KV_HEADS, HEAD_DIM = 32, 8, 128
EPS = 1e-6
ROPE_THETA = 10000.0
N_CORES = 8
TP = 4  # head groups
HQ = NUM_HEADS // TP        # 8 q heads per core
HKV = NUM_KV_HEADS // TP    # 2 kv heads per core
KT = HIDDEN // 128          # 32 k tiles
IT = S // 128               # 16 token tiles
IB = S // 512               # 4 token blocks (512 wide)


def _split_drain_waits():
    """walrus here rejects >1 sync wait on the tail Drain; split them."""
    from concourse import mybir
    from concourse.tile import TileContext
    from concourse.vector_clock import ScopedClock

    def _drain_and_barrier(self, tick_clock, wait_clock):
        drain_inst = self.nc.sync.drain()
        wait_clock.add_sem_waits(
            drain_inst.ins, ScopedClock({None: tick_clock.global_clock})
        )
        inst = drain_inst.ins
        si = inst.sync_info
        if si is not None and si.on_wait is not None and len(si.on_wait) > 1:
            waits = list(si.on_wait)
            del si.on_wait[1:]
            for i in range(1, len(waits)):
                e_inst = self.nc.sync.drain().ins
                if e_inst.sync_info is None:
                    e_inst.sync_info = mybir.SyncInfo(on_wait=[], on_update=[])
                e_inst.sync_info.on_wait.extend(waits[i : i + 1])
        self.nc.all_engine_barrier()
        assert self.sems is not None
        popped = self.nc._tile_sem_poison_stack.pop()
        assert popped is self._sem_poison
        self.nc.clear_and_free_semaphores(list(self.sems.allocated().values()))
        self.nc.all_engine_barrier()
        _fixup_wait_limits(self.nc)

    TileContext._drain_and_barrier = _drain_and_barrier


def _fixup_wait_limits(nc):
    """walrus in this image caps sync waits per instruction (DMA: hit at 3,
    Drain at 4+). Hoist excess waits onto nop instructions inserted just
    before the offender on the same engine (waits still complete before the
    original program point; engine order preserves semantics)."""
    from concourse import mybir

    def limit_for(inst):
        return 1

    def mk_nop(engine):
        bi = nc.engines[engine].nop(nofuse=True)
        inst = bi.ins if hasattr(bi, "ins") else bi
        for f in nc.m.functions:
            for blk in f.blocks:
                if blk.instructions and blk.instructions[-1] is inst:
                    blk.instructions.pop()
        return inst

    for f in nc.m.functions:
        for blk in f.blocks:
            out = []
            for inst in blk.instructions:
                si = inst.sync_info
                nw = len(si.on_wait) if si is not None and si.on_wait else 0
                lim = limit_for(inst)
                if nw > lim:
                    waits = list(si.on_wait)
                    del si.on_wait[lim:]
                    for w in waits[lim:]:
                        nop = mk_nop(inst.engine)
                        nop.sync_info = mybir.SyncInfo(on_wait=[w], on_update=[])
                        out.append(nop)
                out.append(inst)
            blk.instructions[:] = out


def build_bass():
    import concourse.bass as bass
    import concourse.tile as tile
    from concourse import mybir

    _split_drain_waits()

    f32 = mybir.dt.float32
    bf16 = mybir.dt.bfloat16
    AF = mybir.ActivationFunctionType
    ALU = mybir.AluOpType

    nc = bass.Bass("TRN2", target_bir_lowering=False, debug=False)

    hst = nc.dram_tensor("hst", [IT, 128, KT, 128], bf16, kind="ExternalInput")
    wq = nc.dram_tensor("wq", [128, KT, HQ * 128], bf16, kind="ExternalInput")
    wkv = nc.dram_tensor("wkv", [128, KT, 4 * 128], bf16, kind="ExternalInput")
    wo = nc.dram_tensor("wo", [128, HQ, HIDDEN], bf16, kind="ExternalInput")
    cosq = nc.dram_tensor("cosq", [128, IT, 128], bf16, kind="ExternalInput")
    sinq = nc.dram_tensor("sinq", [128, IT, 128], bf16, kind="ExternalInput")
    cosk = nc.dram_tensor("cosk", [128, IT, 128], bf16, kind="ExternalInput")
    sink = nc.dram_tensor("sink", [128, IT, 128], bf16, kind="ExternalInput")
    masks = nc.dram_tensor("masks", [128, 4, 512], bf16, kind="ExternalInput")
    ident = nc.dram_tensor("ident", [128, 128], bf16, kind="ExternalInput")
    out = nc.dram_tensor("out", [S, HIDDEN], f32, kind="ExternalOutput")
    dbg = os.environ.get("K_DEBUG", "") == "1"
    if dbg:
        qT_d = nc.dram_tensor("qT_d", [128, HQ, S], f32, kind="ExternalOutput")
        kT_d = nc.dram_tensor("kT_d", [128, HKV, S], f32, kind="ExternalOutput")
        va_d = nc.dram_tensor("va_d", [128, HKV, IT, 129], f32, kind="ExternalOutput")
        aoT_d = nc.dram_tensor("aoT_d", [128, HQ, S], f32, kind="ExternalOutput")

    with tile.TileContext(nc) as tc, ExitStack() as top:
        const = top.enter_context(tc.tile_pool(name="const", bufs=1))
        res = top.enter_context(tc.tile_pool(name="res", bufs=1))

        cos_sb = {}
        sin_sb = {}
        cos_sb["q"] = const.tile([128, IT, 128], bf16, tag="cosq", name="cosq_sb")
        sin_sb["q"] = const.tile([128, IT, 128], bf16, tag="sinq", name="sinq_sb")
        cos_sb["k"] = const.tile([128, IT, 128], bf16, tag="cosk", name="cosk_sb")
        sin_sb["k"] = const.tile([128, IT, 128], bf16, tag="sink", name="sink_sb")
        nc.sync.dma_start(out=cos_sb["q"], in_=cosq.ap())
        nc.sync.dma_start(out=sin_sb["q"], in_=sinq.ap())
        nc.sync.dma_start(out=cos_sb["k"], in_=cosk.ap())
        nc.sync.dma_start(out=sin_sb["k"], in_=sink.ap())
        masks_sb = const.tile([128, 4, 512], bf16, tag="masks")
        nc.sync.dma_start(out=masks_sb, in_=masks.ap())
        ident_sb = const.tile([128, 128], bf16, tag="ident")
        nc.sync.dma_start(out=ident_sb, in_=ident.ap())
        eps_sb = const.tile([128, 1], f32, tag="eps")
        nc.vector.memset(eps_sb, EPS)

        aoT = res.tile([128, HQ, S], bf16, tag="aoT")
        qkv_stack = ExitStack()
        qkv_res = qkv_stack.enter_context(tc.tile_pool(name="qkv_res", bufs=1))
        qT = qkv_res.tile([128, HQ, S], bf16, tag="qT")
        kT = qkv_res.tile([128, HKV, S], bf16, tag="kT")
        v_aug = qkv_res.tile([128, HKV, IT, 129], bf16, tag="vaug")
        nc.vector.memset(v_aug[:, :, :, 128:129], 1.0)

        def norm_rope_transpose(psum_sl, which, h_off, it, dst, stage, small):
            """psum_sl: [128 i, 128 d] raw projection; writes dst[:, it*128:...]"""
            ss = small.tile([128, 1], f32, tag="ss")
            sq = stage.tile([128, 128], f32, tag="sq")
            nc.scalar.activation(
                out=sq, in_=psum_sl, func=AF.Square, accum_out=ss
            )
            rstd = small.tile([128, 1], f32, tag="rstd")
            nc.scalar.activation(
                out=rstd, in_=ss, func=AF.Sqrt, scale=1.0 / HEAD_DIM, bias=eps_sb
            )
            nc.vector.reciprocal(out=rstd, in_=rstd)
            qn = stage.tile([128, 128], f32, tag="qn")
            nc.scalar.activation(out=qn, in_=psum_sl, func=AF.Copy, scale=rstd)
            cos_t = cos_sb[which][:, it, :]
            sin_t = sin_sb[which][:, it, :]
            ta = stage.tile([128, 64], f32, tag="ta")
            tb = stage.tile([128, 64], f32, tag="tb")
            rq = stage.tile([128, 128], bf16, tag="rq")
            # top: qn_top*cos_top - qn_bot*sin_top
            nc.vector.tensor_mul(out=ta, in0=qn[:, 0:64], in1=cos_t[:, 0:64])
            nc.vector.tensor_mul(out=tb, in0=qn[:, 64:128], in1=sin_t[:, 0:64])
            nc.vector.tensor_sub(out=rq[:, 0:64], in0=ta, in1=tb)
            # bottom: qn_bot*cos_bot + qn_top*sin_bot
            tc_ = stage.tile([128, 64], f32, tag="tc")
            td = stage.tile([128, 64], f32, tag="td")
            nc.vector.tensor_mul(out=tc_, in0=qn[:, 64:128], in1=cos_t[:, 64:128])
            nc.vector.tensor_mul(out=td, in0=qn[:, 0:64], in1=sin_t[:, 64:128])
            nc.vector.tensor_add(out=rq[:, 64:128], in0=tc_, in1=td)
            ps_t = trans_psum.tile([128, 128], bf16, tag="ps_t")
            nc.tensor.transpose(ps_t, rq, ident_sb)
            nc.scalar.activation(
                out=dst[:, h_off, it * 128 : (it + 1) * 128], in_=ps_t, func=AF.Copy
            )

        # ---------------- Phase 1a: Q projection ----------------
        with ExitStack() as p1:
            wpool = p1.enter_context(tc.tile_pool(name="wq", bufs=1))
            hpool = p1.enter_context(tc.tile_pool(name="hst", bufs=2))
            qpsum = p1.enter_context(tc.tile_pool(name="qpsum", bufs=2, space="PSUM"))
            trans_psum = p1.enter_context(
                tc.tile_pool(name="tpsum", bufs=2, space="PSUM")
            )
            stage = p1.enter_context(tc.tile_pool(name="stage", bufs=3))
            small = p1.enter_context(tc.tile_pool(name="small", bufs=4))

            wq_sb = wpool.tile([128, KT, HQ * 128], bf16, tag="wq")
            nc.sync.dma_start(out=wq_sb, in_=wq.ap())
            for it in range(IT):
                ht = hpool.tile([128, KT, 128], bf16, tag="ht")
                nc.sync.dma_start(out=ht, in_=hst.ap()[it])
                ps = [
                    qpsum.tile([128, 512], f32, tag=f"psq{j}", name=f"psq{j}_{it}")
                    for j in range(2)
                ]
                for kt in range(KT):
                    for j in range(2):
                        nc.tensor.matmul(
                            ps[j][:],
                            ht[:, kt, :],
                            wq_sb[:, kt, j * 512 : (j + 1) * 512],
                            start=(kt == 0),
                            stop=(kt == KT - 1),
                        )
                for h in range(HQ):
                    sl = ps[h // 4][:, (h % 4) * 128 : (h % 4) * 128 + 128]
                    norm_rope_transpose(sl, "q", h, it, qT, stage, small)

        # ---------------- Phase 1b: K/V projection ----------------
        with ExitStack() as p1:
            wpool = p1.enter_context(tc.tile_pool(name="wkv", bufs=1))
            hpool = p1.enter_context(tc.tile_pool(name="hst2", bufs=2))
            qpsum = p1.enter_context(tc.tile_pool(name="kvpsum", bufs=2, space="PSUM"))
            trans_psum = p1.enter_context(
                tc.tile_pool(name="tpsum2", bufs=2, space="PSUM")
            )
            stage = p1.enter_context(tc.tile_pool(name="stage2", bufs=3))
            small = p1.enter_context(tc.tile_pool(name="small2", bufs=4))

            wkv_sb = wpool.tile([128, KT, 512], bf16, tag="wkv")
            nc.sync.dma_start(out=wkv_sb, in_=wkv.ap())
            for it in range(IT):
                ht = hpool.tile([128, KT, 128], bf16, tag="ht2")
                nc.sync.dma_start(out=ht, in_=hst.ap()[it])
                pkv = qpsum.tile([128, 512], f32, tag="pskv")
                for kt in range(KT):
                    nc.tensor.matmul(
                        pkv[:],
                        ht[:, kt, :],
                        wkv_sb[:, kt, :],
                        start=(kt == 0),
                        stop=(kt == KT - 1),
                    )
                for g in range(HKV):
                    sl = pkv[:, g * 128 : (g + 1) * 128]
                    norm_rope_transpose(sl, "k", g, it, kT, stage, small)
                for g in range(HKV):
                    sl = pkv[:, 256 + g * 128 : 256 + g * 128 + 128]
                    nc.scalar.activation(
                        out=v_aug[:, g, it, 0:128], in_=sl, func=AF.Copy
                    )

        # ---------------- Phase 2: attention ----------------
        with ExitStack() as p2:
            spsum = p2.enter_context(tc.tile_pool(name="spsum", bufs=2, space="PSUM"))
            opsum = p2.enter_context(tc.tile_pool(name="opsum", bufs=1, space="PSUM"))
            trans_psum = p2.enter_context(
                tc.tile_pool(name="tpsum3", bufs=2, space="PSUM")
            )
            ptpool = p2.enter_context(tc.tile_pool(name="pt", bufs=2))
            stage = p2.enter_context(tc.tile_pool(name="stage3", bufs=3))
            small = p2.enter_context(tc.tile_pool(name="small3", bufs=4))

            for h in range(HQ):
                g = h // (HQ // HKV)
                for ib in range(IB):
                    npt = 4 * ib + 4
                    pts = []
                    for jt in range(npt):
                        ps_s = spsum.tile([128, 512], f32, tag="ps_s")
                        nc.tensor.matmul(
                            ps_s[:],
                            kT[:, g, jt * 128 : (jt + 1) * 128],
                            qT[:, h, ib * 512 : (ib + 1) * 512],
                            start=True,
                            stop=True,
                        )
                        pt_t = ptpool.tile([128, 512], bf16, tag=f"pt{jt}")
                        nc.scalar.activation(out=pt_t, in_=ps_s, func=AF.Exp)
                        if jt >= 4 * ib:
                            r = jt - 4 * ib
                            nc.vector.tensor_mul(
                                out=pt_t, in0=pt_t, in1=masks_sb[:, r, :]
                            )
                        pts.append(pt_t)
                    for itl in range(4):
                        it_g = ib * 4 + itl
                        po = opsum.tile([128, 129], f32, tag=f"po{itl}")
                        for jt in range(it_g + 1):
                            nc.tensor.matmul(
                                po[:],
                                pts[jt][:, itl * 128 : (itl + 1) * 128],
                                v_aug[:, g, jt, :],
                                start=(jt == 0),
                                stop=(jt == it_g),
                            )
                        rec = small.tile([128, 1], f32, tag="rec")
                        nc.vector.reciprocal(out=rec, in_=po[:, 128:129])
                        ao = stage.tile([128, 128], bf16, tag="ao")
                        nc.scalar.activation(
                            out=ao, in_=po[:, 0:128], func=AF.Copy, scale=rec
                        )
                        ps_t = trans_psum.tile([128, 128], bf16, tag="ps_t2")
                        nc.tensor.transpose(ps_t, ao, ident_sb)
                        nc.scalar.activation(
                            out=aoT[:, h, it_g * 128 : (it_g + 1) * 128],
                            in_=ps_t,
                            func=AF.Copy,
                        )

        if dbg:
            with tc.tile_pool(name="dbgp", bufs=2) as dbgp:
                for (src_t, dst_t, nm) in ((qT, qT_d, "q"), (kT, kT_d, "k"), (aoT, aoT_d, "a")):
                    n_h = src_t.shape[1]
                    for hh in range(n_h):
                        dt_ = dbgp.tile([128, S], f32, tag="dbg", name=f"dbg_{nm}_{hh}")
                        nc.vector.tensor_copy(out=dt_, in_=src_t[:, hh, :])
                        nc.gpsimd.dma_start(out=dst_t.ap()[:, hh, :], in_=dt_)
                for g_ in range(HKV):
                    dt_ = dbgp.tile([128, IT * 129], f32, tag="dbg", name=f"dbg_v_{g_}")
                    nc.vector.tensor_copy(out=dt_, in_=v_aug[:, g_, :, :].rearrange("p a b -> p (a b)"))
                    nc.gpsimd.dma_start(out=va_d.ap()[:, g_, :, :].rearrange("p a b -> p (a b)"), in_=dt_)

        qkv_stack.close()

        # ---------------- Phase 3: output projection ----------------
        with ExitStack() as p3:
            wpool = p3.enter_context(tc.tile_pool(name="wo", bufs=1))
            opsum3 = p3.enter_context(tc.tile_pool(name="opsum3", bufs=2, space="PSUM"))
            ostage = p3.enter_context(tc.tile_pool(name="ostage", bufs=2))

            wo_sb = wpool.tile([128, HQ, HIDDEN], bf16, tag="wo")
            nc.sync.dma_start(out=wo_sb, in_=wo.ap())
            for m in range(IT):
                st = ostage.tile([128, HIDDEN], f32, tag="ost")
                for half in range(2):
                    pos = [
                        opsum3.tile(
                            [128, 512], f32, tag=f"po3{i}", name=f"po3_{m}_{half}_{i}"
                        )
                        for i in range(4)
                    ]
                    for k in range(HQ):
                        for i4 in range(4):
                            nc.tensor.matmul(
                                pos[i4][:],
                                aoT[:, k, m * 128 : (m + 1) * 128],
                                wo_sb[:, k, half * 2048 + i4 * 512 : half * 2048 + (i4 + 1) * 512],
                                start=(k == 0),
                                stop=(k == HQ - 1),
                            )
                    for i4 in range(4):
                        nc.scalar.activation(
                            out=st[:, half * 2048 + i4 * 512 : half * 2048 + (i4 + 1) * 512],
                            in_=pos[i4],
                            func=AF.Copy,
                        )
                nc.sync.dma_start(
                    out=out.ap()[m * 128 : (m + 1) * 128, :], in_=st
                )

    return nc


def prep_core_inputs(hidden_states, position_ids, Wq, Wk, Wv, Wo, q_norm_w, k_norm_w):
    """Host-side shard + layout prep. Returns list of 8 in_maps."""
    pos = np.asarray(position_ids).reshape(-1).astype(np.float64)  # [S]
    inv_freq = 1.0 / (
        ROPE_THETA ** (np.arange(0, HEAD_DIM, 2, dtype=np.float64) / HEAD_DIM)
    )  # [64]
    ang = pos[:, None] * inv_freq[None, :]  # [S, 64]
    emb = np.concatenate([ang, ang], axis=1)  # [S, 128]
    scale = HEAD_DIM ** (-0.25)
    cos = (np.cos(emb) * scale).astype(np.float32)  # [S, 128]
    sin = (np.sin(emb) * scale).astype(np.float32)
    qw = np.asarray(q_norm_w, dtype=np.float32)
    kw = np.asarray(k_norm_w, dtype=np.float32)
    qw_roll = np.concatenate([qw[64:], qw[:64]])
    kw_roll = np.concatenate([kw[64:], kw[:64]])

    def table(t):  # [S,128] -> [128, IT, 128]
        return np.ascontiguousarray(
            t.reshape(IT, 128, 128).transpose(1, 0, 2)
        )

    cosq_t = table(cos * qw[None, :]).astype(BF16)
    sinq_t = table(sin * qw_roll[None, :]).astype(BF16)
    cosk_t = table(cos * kw[None, :]).astype(BF16)
    sink_t = table(sin * kw_roll[None, :]).astype(BF16)

    # causal masks for the 4 diagonal offsets
    jj = np.arange(128)[:, None]
    ii = np.arange(512)[None, :]
    masks = np.stack(
        [(jj <= ii - 128 * r).astype(np.float32) for r in range(4)]
    ).transpose(1, 0, 2)  # [128, 4, 512]
    masks = masks.astype(BF16)
    ident = np.eye(128, dtype=np.float32).astype(BF16)

    hs = np.asarray(hidden_states, dtype=np.float32)
    Wq = np.asarray(Wq, dtype=np.float32)
    Wk = np.asarray(Wk, dtype=np.float32)
    Wv = np.asarray(Wv, dtype=np.float32)
    Wo = np.asarray(Wo, dtype=np.float32)

    hst_b = []
    for b in range(B):
        hsT = hs[b].T.astype(BF16)  # [4096, 2048]
        # -> [IT, 128(i), KT, 128(k)]: hst[it, ip, kt, kp] = hsT[kt*128+kp, it*128+ip]
        t = hsT.reshape(KT, 128, IT, 128).transpose(2, 1, 0, 3)
        hst_b.append(np.ascontiguousarray(t))

    in_maps = []
    for c in range(N_CORES):
        b, grp = divmod(c, TP)
        wq_s = Wq[:, grp * HQ * 128 : (grp + 1) * HQ * 128].astype(BF16)
        wq_t = np.ascontiguousarray(
            wq_s.reshape(KT, 128, HQ * 128).transpose(1, 0, 2)
        )  # [128, KT, 1024]
        wk_s = Wk[:, grp * HKV * 128 : (grp + 1) * HKV * 128]
        wv_s = Wv[:, grp * HKV * 128 : (grp + 1) * HKV * 128]
        wkv_s = np.concatenate([wk_s, wv_s], axis=1).astype(BF16)  # [4096, 512]
        wkv_t = np.ascontiguousarray(
            wkv_s.reshape(KT, 128, 512).transpose(1, 0, 2)
        )  # [128, KT, 512]
        wo_s = Wo[grp * HQ * 128 : (grp + 1) * HQ * 128, :].astype(BF16)  # [1024, 4096]
        wo_t = np.ascontiguousarray(
            wo_s.reshape(HQ, 128, HIDDEN).transpose(1, 0, 2)
        )  # [128, HQ, 4096]
        in_maps.append(
            {
                "hst": hst_b[b],
                "wq": wq_t,
                "wkv": wkv_t,
                "wo": wo_t,
                "cosq": cosq_t,
                "sinq": sinq_t,
                "cosk": cosk_t,
                "sink": sink_t,
                "masks": masks,
                "ident": ident,
            }
        )
    return in_maps


def kernel(hidden_states, position_ids, Wq, Wk, Wv, Wo, q_norm_w, k_norm_w,
           _trace=False, _tmpdir=None):
    from concourse.bass_utils import run_bass_kernel_spmd

    nc = build_bass()
    in_maps = prep_core_inputs(
        hidden_states, position_ids, Wq, Wk, Wv, Wo, q_norm_w, k_norm_w
    )
    kwargs = {}
    if _trace:
        kwargs = dict(trace=True, tmpdir=_tmpdir)
    res = run_bass_kernel_spmd(nc, in_maps, list(range(N_CORES)), **kwargs)
    partials = [res.results[c]["out"] for c in range(N_CORES)]
    outb = [
        np.sum([partials[b * TP + g] for g in range(TP)], axis=0, dtype=np.float32)
        for b in range(B)
    ]
    full = np.stack(outb).astype(np.float32)  # [2, 2048, 4096]
    if _trace:
        kernel._last_result = res
    return full


# revision 17
# speedup vs baseline: 1.2107x; 1.2107x over previous
"""GQA attention layer (dense_transformer) on 8 Trainium2 NeuronCores.

Sharding: data-parallel over batch (2) x tensor-parallel over head groups (4).
Core c handles batch c//4 and head-group c%4 (8 q heads, 2 kv heads).
Each core computes a partial output (its heads' contribution through its
Wo row-slice); the host sums the 4 partials per batch.

Per-core pipeline (all matmuls bf16, fp32 accumulation):
  P1a: q = hs @ Wq_shard  -> RMSNorm -> RoPE -> transpose -> qT [d, i]
  P1b: k,v = hs @ Wkv_shard -> (k: norm+rope+transpose -> kT), (v: v_aug
       token-major with a ones column appended for softmax denominators)
  P2:  per head: scoresT[j,i] = kT_tile.T @ qT (pre-scaled via tables),
       exp on ScalarE, causal mask on diagonal tiles, PV with v_aug gives
       attn_out[i,d] AND row sums in column 128; normalize by reciprocal
       of the sums during eviction; transpose -> aoT [d, i]
  P3:  out_partial = aoT.T @ Wo_shard
"""
import math
import os
import sys
from contextlib import ExitStack

import numpy as np

_REPO = "/opt/trn_rl_repo"
_PKGS = "/opt/pypackages"
for _p in (_REPO, _PKGS):
    if _p not in sys.path:
        sys.path.append(_p)

import ml_dtypes

BF16 = ml_dtypes.bfloat16

B, S, HIDDEN = 2, 2048, 4096
NUM_HEADS, NUM_KV_HEADS, HEAD_DIM = 32, 8, 128
EPS = 1e-6
ROPE_THETA = 10000.0
N_CORES = 8
TP = 4  # head groups
HQ = NUM_HEADS // TP        # 8 q heads per core
HKV = NUM_KV_HEADS // TP    # 2 kv heads per core
KT = HIDDEN // 128          # 32 k tiles
IT = S // 128               # 16 token tiles
IB = S // 512               # 4 token blocks (512 wide)


def _split_drain_waits():
    """walrus here rejects >1 sync wait on the tail Drain; split them."""
    from concourse import mybir
    from concourse.tile import TileContext
    from concourse.vector_clock import ScopedClock

    def _drain_and_barrier(self, tick_clock, wait_clock):
        drain_inst = self.nc.sync.drain()
        wait_clock.add_sem_waits(
            drain_inst.ins, ScopedClock({None: tick_clock.global_clock})
        )
        inst = drain_inst.ins
        si = inst.sync_info
        if si is not None and si.on_wait is not None and len(si.on_wait) > 1:
            waits = list(si.on_wait)
            del si.on_wait[1:]
            for i in range(1, len(waits)):
                e_inst = self.nc.sync.drain().ins
                if e_inst.sync_info is None:
                    e_inst.sync_info = mybir.SyncInfo(on_wait=[], on_update=[])
                e_inst.sync_info.on_wait.extend(waits[i : i + 1])
        self.nc.all_engine_barrier()
        assert self.sems is not None
        popped = self.nc._tile_sem_poison_stack.pop()
        assert popped is self._sem_poison
        self.nc.clear_and_free_semaphores(list(self.sems.allocated().values()))
        self.nc.all_engine_barrier()
        _fixup_wait_limits(self.nc)

    TileContext._drain_and_barrier = _drain_and_barrier


def _fixup_wait_limits(nc):
    """walrus in this image caps sync waits per instruction (DMA: hit at 3,
    Drain at 4+). Hoist excess waits onto nop instructions inserted just
    before the offender on the same engine (waits still complete before the
    original program point; engine order preserves semantics)."""
    from concourse import mybir

    def limit_for(inst):
        return 1

    def mk_nop(engine):
        bi = nc.engines[engine].nop(nofuse=True)
        inst = bi.ins if hasattr(bi, "ins") else bi
        for f in nc.m.functions:
            for blk in f.blocks:
                if blk.instructions and blk.instructions[-1] is inst:
                    blk.instructions.pop()
        return inst

    for f in nc.m.functions:
        for blk in f.blocks:
            out = []
            for inst in blk.instructions:
                si = inst.sync_info
                nw = len(si.on_wait) if si is not None and si.on_wait else 0
                lim = limit_for(inst)
                if nw > lim:
                    waits = list(si.on_wait)
                    del si.on_wait[lim:]
                    for w in waits[lim:]:
                        nop = mk_nop(inst.engine)
                        nop.sync_info = mybir.SyncInfo(on_wait=[w], on_update=[])
                        out.append(nop)
                out.append(inst)
            blk.instructions[:] = out


def build_bass():
    import concourse.bass as bass
    import concourse.tile as tile
    from concourse import mybir

    _split_drain_waits()

    f32 = mybir.dt.float32
    bf16 = mybir.dt.bfloat16
    AF = mybir.ActivationFunctionType
    ALU = mybir.AluOpType

    nc = bass.Bass("TRN2", target_bir_lowering=False, debug=False)

    hst = nc.dram_tensor("hst", [IT, 128, KT, 128], bf16, kind="ExternalInput")
    wq = nc.dram_tensor("wq", [128, KT, HQ * 128], bf16, kind="ExternalInput")
    wkv = nc.dram_tensor("wkv", [128, KT, 4 * 128], bf16, kind="ExternalInput")
    wo = nc.dram_tensor("wo", [128, HQ, HIDDEN], bf16, kind="ExternalInput")
    cosq = nc.dram_tensor("cosq", [128, IT, 128], bf16, kind="ExternalInput")
    sinq = nc.dram_tensor("sinq", [128, IT, 128], bf16, kind="ExternalInput")
    cosk = nc.dram_tensor("cosk", [128, IT, 128], bf16, kind="ExternalInput")
    sink = nc.dram_tensor("sink", [128, IT, 128], bf16, kind="ExternalInput")
    masks = nc.dram_tensor("masks", [128, 4, 512], bf16, kind="ExternalInput")
    ident = nc.dram_tensor("ident", [128, 128], bf16, kind="ExternalInput")
    out = nc.dram_tensor("out", [S, HIDDEN], f32, kind="ExternalOutput")
    dbg = os.environ.get("K_DEBUG", "") == "1"
    if dbg:
        qT_d = nc.dram_tensor("qT_d", [128, HQ, S], f32, kind="ExternalOutput")
        kT_d = nc.dram_tensor("kT_d", [128, HKV, S], f32, kind="ExternalOutput")
        va_d = nc.dram_tensor("va_d", [128, HKV, IT, 129], f32, kind="ExternalOutput")
        aoT_d = nc.dram_tensor("aoT_d", [128, HQ, S], f32, kind="ExternalOutput")

    with tile.TileContext(nc) as tc, ExitStack() as top:
        const = top.enter_context(tc.tile_pool(name="const", bufs=1))
        res = top.enter_context(tc.tile_pool(name="res", bufs=1))

        cos_sb = {}
        sin_sb = {}
        cos_sb["q"] = const.tile([128, IT, 128], bf16, tag="cosq", name="cosq_sb")
        sin_sb["q"] = const.tile([128, IT, 128], bf16, tag="sinq", name="sinq_sb")
        cos_sb["k"] = const.tile([128, IT, 128], bf16, tag="cosk", name="cosk_sb")
        sin_sb["k"] = const.tile([128, IT, 128], bf16, tag="sink", name="sink_sb")
        nc.sync.dma_start(out=cos_sb["q"], in_=cosq.ap())
        nc.sync.dma_start(out=sin_sb["q"], in_=sinq.ap())
        nc.sync.dma_start(out=cos_sb["k"], in_=cosk.ap())
        nc.sync.dma_start(out=sin_sb["k"], in_=sink.ap())
        masks_sb = const.tile([128, 4, 512], bf16, tag="masks")
        nc.sync.dma_start(out=masks_sb, in_=masks.ap())
        ident_sb = const.tile([128, 128], bf16, tag="ident")
        nc.sync.dma_start(out=ident_sb, in_=ident.ap())
        eps_sb = const.tile([128, 1], f32, tag="eps")
        nc.vector.memset(eps_sb, EPS)

        qkv_stack = ExitStack()
        qkv_res = qkv_stack.enter_context(tc.tile_pool(name="qkv_res", bufs=1))
        qT = qkv_res.tile([128, HQ, S], bf16, tag="qT")
        kT = qkv_res.tile([128, HKV, S], bf16, tag="kT")
        v_aug = qkv_res.tile([128, HKV, IT, 129], bf16, tag="vaug")
        nc.vector.memset(v_aug[:, :, :, 128:129], 1.0)

        def rms_rope_group(psum_t, n_heads, which, h_base, it, dst, stage, small):
            """psum_t: [128 i, n_heads*128] raw projections (PSUM).
            Normalizes each 128-wide head group, applies RoPE (tables carry
            the 128^-0.25 score scale and the norm weight), transposes each
            head to [d, i] and writes dst[:, h_base+h, it*128:...]."""
            w = n_heads * 128
            sq = stage.tile([128, 512], f32, tag="sq", name=f"sq_{which}_{it}_{h_base}")
            nc.scalar.activation(out=sq[:, 0:w], in_=psum_t, func=AF.Square)
            ss = small.tile([128, 4], f32, tag="ss", name=f"ss_{which}_{it}_{h_base}")
            nc.vector.tensor_reduce(
                out=ss[:, 0:n_heads],
                in_=sq[:, 0:w].rearrange("p (h d) -> p h d", h=n_heads),
                op=ALU.add, axis=mybir.AxisListType.X,
            )
            rstd = small.tile([128, 4], f32, tag="rstd", name=f"rstd_{which}_{it}_{h_base}")
            nc.scalar.activation(
                out=rstd[:, 0:n_heads], in_=ss[:, 0:n_heads], func=AF.Sqrt,
                scale=1.0 / HEAD_DIM, bias=eps_sb,
            )
            nc.vector.reciprocal(out=rstd[:, 0:n_heads], in_=rstd[:, 0:n_heads])
            qn = stage.tile([128, 512], f32, tag="qn", name=f"qn_{which}_{it}_{h_base}")
            for h in range(n_heads):
                nc.vector.tensor_scalar_mul(
                    out=qn[:, h * 128 : (h + 1) * 128],
                    in0=psum_t[:, h * 128 : (h + 1) * 128],
                    scalar1=rstd[:, h : h + 1],
                )
            qn3 = qn[:, 0:w].rearrange("p (h d) -> p h d", h=n_heads)
            cos_t = cos_sb[which][:, it, :]
            sin_t = sin_sb[which][:, it, :]
            ct = cos_t[:, 0:64][:, None, :].broadcast_to([128, n_heads, 64])
            cb = cos_t[:, 64:128][:, None, :].broadcast_to([128, n_heads, 64])
            st_ = sin_t[:, 0:64][:, None, :].broadcast_to([128, n_heads, 64])
            sb_ = sin_t[:, 64:128][:, None, :].broadcast_to([128, n_heads, 64])
            ta = stage.tile([128, 4, 64], f32, tag="ta", name=f"ta_{which}_{it}_{h_base}")
            tb = stage.tile([128, 4, 64], f32, tag="tb", name=f"tb_{which}_{it}_{h_base}")
            rq = stage.tile([128, 512], bf16, tag="rq", name=f"rq_{which}_{it}_{h_base}")
            rq3 = rq[:, 0:w].rearrange("p (h d) -> p h d", h=n_heads)
            nc.vector.tensor_mul(out=ta[:, 0:n_heads], in0=qn3[:, :, 0:64], in1=ct)
            nc.vector.tensor_mul(out=tb[:, 0:n_heads], in0=qn3[:, :, 64:128], in1=st_)
            nc.vector.tensor_sub(out=rq3[:, :, 0:64], in0=ta[:, 0:n_heads], in1=tb[:, 0:n_heads])
            nc.vector.tensor_mul(out=ta[:, 0:n_heads], in0=qn3[:, :, 64:128], in1=cb)
            nc.vector.tensor_mul(out=tb[:, 0:n_heads], in0=qn3[:, :, 0:64], in1=sb_)
            nc.vector.tensor_add(out=rq3[:, :, 64:128], in0=ta[:, 0:n_heads], in1=tb[:, 0:n_heads])
            for h in range(n_heads):
                ps_t = trans_psum.tile(
                    [128, 128], bf16, tag="ps_t", name=f"ps_t_{which}_{it}_{h_base+h}"
                )
                nc.tensor.transpose(ps_t, rq[:, h * 128 : (h + 1) * 128], ident_sb)
                nc.vector.tensor_copy(
                    out=dst[:, h_base + h, it * 128 : (it + 1) * 128], in_=ps_t
                )

        # ---------------- Phase 1a: Q projection ----------------
        with ExitStack() as p1:
            wpool = p1.enter_context(tc.tile_pool(name="wq", bufs=1))
            hpool = p1.enter_context(tc.tile_pool(name="hst", bufs=2))
            qpsum = p1.enter_context(tc.tile_pool(name="qpsum", bufs=2, space="PSUM"))
            trans_psum = p1.enter_context(
                tc.tile_pool(name="tpsum", bufs=2, space="PSUM")
            )
            stage = p1.enter_context(tc.tile_pool(name="stage", bufs=2))
            small = p1.enter_context(tc.tile_pool(name="small", bufs=4))

            wq_sb = wpool.tile([128, KT, HQ * 128], bf16, tag="wq")
            nc.sync.dma_start(out=wq_sb, in_=wq.ap())
            for it in range(IT):
                ht = hpool.tile([128, KT, 128], bf16, tag="ht")
                nc.sync.dma_start(out=ht, in_=hst.ap()[it])
                ps = [
                    qpsum.tile([128, 512], f32, tag=f"psq{j}", name=f"psq{j}_{it}")
                    for j in range(2)
                ]
                for kt in range(KT):
                    st = kt == 0
                    sp = kt == KT - 1
                    nc.tensor.matmul(ps[0][:], ht[:, kt, :], wq_sb[:, kt, 0:512],
                                     start=st, stop=sp)
                    nc.tensor.matmul(ps[1][:], ht[:, kt, :], wq_sb[:, kt, 512:1024],
                                     start=st, stop=sp)
                rms_rope_group(ps[0][:, :], 4, "q", 0, it, qT, stage, small)
                rms_rope_group(ps[1][:, :], 4, "q", 4, it, qT, stage, small)

        # ---------------- Phase 1b: K/V projection ----------------
        with ExitStack() as p1:
            wpool = p1.enter_context(tc.tile_pool(name="wkvp", bufs=1))
            hpool = p1.enter_context(tc.tile_pool(name="hst2", bufs=2))
            qpsum = p1.enter_context(tc.tile_pool(name="kvpsum", bufs=2, space="PSUM"))
            trans_psum = p1.enter_context(
                tc.tile_pool(name="tpsum2", bufs=2, space="PSUM")
            )
            stage = p1.enter_context(tc.tile_pool(name="stage2", bufs=2))
            small = p1.enter_context(tc.tile_pool(name="small2", bufs=4))

            wkv_sb = wpool.tile([128, KT, 512], bf16, tag="wkv")
            nc.sync.dma_start(out=wkv_sb, in_=wkv.ap())
            for it in range(IT):
                ht = hpool.tile([128, KT, 128], bf16, tag="ht2")
                nc.sync.dma_start(out=ht, in_=hst.ap()[it])
                pkv = qpsum.tile([128, 512], f32, tag="pskv", name=f"pskv_{it}")
                for kt in range(KT):
                    nc.tensor.matmul(pkv[:], ht[:, kt, :], wkv_sb[:, kt, :],
                                     start=(kt == 0), stop=(kt == KT - 1))
                rms_rope_group(pkv[:, 0:256], 2, "k", 0, it, kT, stage, small)
                for g in range(HKV):
                    sl = pkv[:, 256 + g * 128 : 256 + g * 128 + 128]
                    nc.scalar.activation(
                        out=v_aug[:, g, it, 0:128], in_=sl, func=AF.Copy
                    )

        # ---------------- Phase 2: attention ----------------
        aoT = res.tile([128, HQ, S], bf16, tag="aoT")
        with ExitStack() as p2:
            spsum = p2.enter_context(tc.tile_pool(name="spsum", bufs=2, space="PSUM"))
            opsum = p2.enter_context(tc.tile_pool(name="opsum", bufs=2, space="PSUM"))
            trans_psum = p2.enter_context(
                tc.tile_pool(name="tpsum3", bufs=2, space="PSUM")
            )
            ptpool = p2.enter_context(tc.tile_pool(name="pt", bufs=2))
            stage = p2.enter_context(tc.tile_pool(name="stage3", bufs=3))
            small = p2.enter_context(tc.tile_pool(name="small3", bufs=4))

            for h in range(HQ):
                g = h // (HQ // HKV)
                for ib in range(IB):
                    npt = 4 * ib + 4
                    pts = {}
                    for jp in range(npt // 2):
                        ps_s = spsum.tile(
                            [128, 1024], f32, tag="ps_s", name=f"ps_s_{h}_{ib}_{jp}"
                        )
                        for half in range(2):
                            jt = jp * 2 + half
                            nc.tensor.matmul(
                                ps_s[:, half * 512 : (half + 1) * 512],
                                kT[:, g, jt * 128 : (jt + 1) * 128],
                                qT[:, h, ib * 512 : (ib + 1) * 512],
                                start=True,
                                stop=True,
                            )
                        pt_t = ptpool.tile(
                            [128, 1024], bf16, tag=f"pt{jp}", name=f"pt_{h}_{ib}_{jp}"
                        )
                        nc.scalar.activation(out=pt_t, in_=ps_s, func=AF.Exp)
                        for half in range(2):
                            jt = jp * 2 + half
                            if jt >= 4 * ib:
                                r = jt - 4 * ib
                                nc.vector.tensor_mul(
                                    out=pt_t[:, half * 512 : (half + 1) * 512],
                                    in0=pt_t[:, half * 512 : (half + 1) * 512],
                                    in1=masks_sb[:, r, :],
                                )
                            pts[jt] = pt_t[:, half * 512 : (half + 1) * 512]
                    for itl in range(4):
                        it_g = ib * 4 + itl
                        po = opsum.tile(
                            [128, 129], f32, tag="po", name=f"po_{h}_{ib}_{itl}"
                        )
                        for jt in range(it_g + 1):
                            nc.tensor.matmul(
                                po[:],
                                pts[jt][:, itl * 128 : (itl + 1) * 128],
                                v_aug[:, g, jt, :],
                                start=(jt == 0),
                                stop=(jt == it_g),
                            )
                        rec = small.tile([128, 1], f32, tag="rec", name=f"rec_{h}_{it_g}")
                        nc.vector.reciprocal(out=rec, in_=po[:, 128:129])
                        ao = stage.tile([128, 128], bf16, tag="ao", name=f"ao_{h}_{it_g}")
                        nc.vector.tensor_scalar_mul(
                            out=ao, in0=po[:, 0:128], scalar1=rec
                        )
                        ps_t = trans_psum.tile(
                            [128, 128], bf16, tag="ps_t2", name=f"ps_t2_{h}_{it_g}"
                        )
                        nc.tensor.transpose(ps_t, ao, ident_sb)
                        nc.vector.tensor_copy(
                            out=aoT[:, h, it_g * 128 : (it_g + 1) * 128], in_=ps_t
                        )

        if dbg:
            with tc.tile_pool(name="dbgp", bufs=2) as dbgp:
                for (src_t, dst_t, nm) in ((qT, qT_d, "q"), (kT, kT_d, "k"), (aoT, aoT_d, "a")):
                    n_h = src_t.shape[1]
                    for hh in range(n_h):
                        dt_ = dbgp.tile([128, S], f32, tag="dbg", name=f"dbg_{nm}_{hh}")
                        nc.vector.tensor_copy(out=dt_, in_=src_t[:, hh, :])
                        nc.gpsimd.dma_start(out=dst_t.ap()[:, hh, :], in_=dt_)
                for g_ in range(HKV):
                    dt_ = dbgp.tile([128, IT * 129], f32, tag="dbg", name=f"dbg_v_{g_}")
                    nc.vector.tensor_copy(out=dt_, in_=v_aug[:, g_, :, :].rearrange("p a b -> p (a b)"))
                    nc.gpsimd.dma_start(out=va_d.ap()[:, g_, :, :].rearrange("p a b -> p (a b)"), in_=dt_)

        qkv_stack.close()

        # ---------------- Phase 3: output projection ----------------
        with ExitStack() as p3:
            wpool = p3.enter_context(tc.tile_pool(name="wo", bufs=1))
            opsum3 = p3.enter_context(tc.tile_pool(name="opsum3", bufs=2, space="PSUM"))
            ostage = p3.enter_context(tc.tile_pool(name="ostage", bufs=2))

            wo_sb = wpool.tile([128, HQ, HIDDEN], bf16, tag="wo")
            nc.sync.dma_start(out=wo_sb, in_=wo.ap())
            for m in range(IT):
                st = ostage.tile([128, HIDDEN], f32, tag="ost")
                for half in range(2):
                    pos = [
                        opsum3.tile(
                            [128, 512], f32, tag=f"po3{i}", name=f"po3_{m}_{half}_{i}"
                        )
                        for i in range(4)
                    ]
                    for k in range(HQ):
                        for i4 in range(4):
                            nc.tensor.matmul(
                                pos[i4][:],
                                aoT[:, k, m * 128 : (m + 1) * 128],
                                wo_sb[:, k, half * 2048 + i4 * 512 : half * 2048 + (i4 + 1) * 512],
                                start=(k == 0),
                                stop=(k == HQ - 1),
                            )
                    for i4 in range(4):
                        nc.scalar.activation(
                            out=st[:, half * 2048 + i4 * 512 : half * 2048 + (i4 + 1) * 512],
                            in_=pos[i4],
                            func=AF.Copy,
                        )
                nc.sync.dma_start(
                    out=out.ap()[m * 128 : (m + 1) * 128, :], in_=st
                )

    return nc


def prep_core_inputs(hidden_states, position_ids, Wq, Wk, Wv, Wo, q_norm_w, k_norm_w):
    """Host-side shard + layout prep. Returns list of 8 in_maps."""
    pos = np.asarray(position_ids).reshape(-1).astype(np.float64)  # [S]
    inv_freq = 1.0 / (
        ROPE_THETA ** (np.arange(0, HEAD_DIM, 2, dtype=np.float64) / HEAD_DIM)
    )  # [64]
    ang = pos[:, None] * inv_freq[None, :]  # [S, 64]
    emb = np.concatenate([ang, ang], axis=1)  # [S, 128]
    scale = HEAD_DIM ** (-0.25)
    cos = (np.cos(emb) * scale).astype(np.float32)  # [S, 128]
    sin = (np.sin(emb) * scale).astype(np.float32)
    qw = np.asarray(q_norm_w, dtype=np.float32)
    kw = np.asarray(k_norm_w, dtype=np.float32)
    qw_roll = np.concatenate([qw[64:], qw[:64]])
    kw_roll = np.concatenate([kw[64:], kw[:64]])

    def table(t):  # [S,128] -> [128, IT, 128]
        return np.ascontiguousarray(
            t.reshape(IT, 128, 128).transpose(1, 0, 2)
        )

    cosq_t = table(cos * qw[None, :]).astype(BF16)
    sinq_t = table(sin * qw_roll[None, :]).astype(BF16)
    cosk_t = table(cos * kw[None, :]).astype(BF16)
    sink_t = table(sin * kw_roll[None, :]).astype(BF16)

    # causal masks for the 4 diagonal offsets
    jj = np.arange(128)[:, None]
    ii = np.arange(512)[None, :]
    masks = np.stack(
        [(jj <= ii - 128 * r).astype(np.float32) for r in range(4)]
    ).transpose(1, 0, 2)  # [128, 4, 512]
    masks = masks.astype(BF16)
    ident = np.eye(128, dtype=np.float32).astype(BF16)

    hs = np.asarray(hidden_states, dtype=np.float32)
    Wq = np.asarray(Wq, dtype=np.float32)
    Wk = np.asarray(Wk, dtype=np.float32)
    Wv = np.asarray(Wv, dtype=np.float32)
    Wo = np.asarray(Wo, dtype=np.float32)

    hst_b = []
    for b in range(B):
        hsT = hs[b].T.astype(BF16)  # [4096, 2048]
        # -> [IT, 128(i), KT, 128(k)]: hst[it, ip, kt, kp] = hsT[kt*128+kp, it*128+ip]
        t = hsT.reshape(KT, 128, IT, 128).transpose(2, 1, 0, 3)
        hst_b.append(np.ascontiguousarray(t))

    in_maps = []
    for c in range(N_CORES):
        b, grp = divmod(c, TP)
        wq_s = Wq[:, grp * HQ * 128 : (grp + 1) * HQ * 128].astype(BF16)
        wq_t = np.ascontiguousarray(
            wq_s.reshape(KT, 128, HQ * 128).transpose(1, 0, 2)
        )  # [128, KT, 1024]
        wk_s = Wk[:, grp * HKV * 128 : (grp + 1) * HKV * 128]
        wv_s = Wv[:, grp * HKV * 128 : (grp + 1) * HKV * 128]
        wkv_s = np.concatenate([wk_s, wv_s], axis=1).astype(BF16)  # [4096, 512]
        wkv_t = np.ascontiguousarray(
            wkv_s.reshape(KT, 128, 512).transpose(1, 0, 2)
        )  # [128, KT, 512]
        wo_s = Wo[grp * HQ * 128 : (grp + 1) * HQ * 128, :].astype(BF16)  # [1024, 4096]
        wo_t = np.ascontiguousarray(
            wo_s.reshape(HQ, 128, HIDDEN).transpose(1, 0, 2)
        )  # [128, HQ, 4096]
        in_maps.append(
            {
                "hst": hst_b[b],
                "wq": wq_t,
                "wkv": wkv_t,
                "wo": wo_t,
                "cosq": cosq_t,
                "sinq": sinq_t,
                "cosk": cosk_t,
                "sink": sink_t,
                "masks": masks,
                "ident": ident,
            }
        )
    return in_maps


def kernel(hidden_states, position_ids, Wq, Wk, Wv, Wo, q_norm_w, k_norm_w,
           _trace=False, _tmpdir=None):
    from concourse.bass_utils import run_bass_kernel_spmd

    nc = build_bass()
    in_maps = prep_core_inputs(
        hidden_states, position_ids, Wq, Wk, Wv, Wo, q_norm_w, k_norm_w
    )
    kwargs = {}
    if _trace:
        kwargs = dict(trace=True, tmpdir=_tmpdir)
    res = run_bass_kernel_spmd(nc, in_maps, list(range(N_CORES)), **kwargs)
    partials = [res.results[c]["out"] for c in range(N_CORES)]
    outb = [
        np.sum([partials[b * TP + g] for g in range(TP)], axis=0, dtype=np.float32)
        for b in range(B)
    ]
    full = np.stack(outb).astype(np.float32)  # [2, 2048, 4096]
    if _trace:
        kernel._last_result = res
    return full
